# revision 1
# baseline (speedup 1.0000x reference)
"""Multi-head attention Bass/Tile kernel for Trainium2 (8 NeuronCores, SPMD).

Reference semantics (note the reference's intentional name swap):
    k = split_heads(query @ Wk.T); q = split_heads(key @ Wq.T)
    v = split_heads(value @ Wv.T)
    wei = (q @ k^T) * C**-0.5
    wei = where(mask == 0, 0.0, wei)        # masked scores become 0, NOT -inf
    wei = softmax(wei, axis=-1)             # masked entries contribute exp(0)=1
    out = (wei @ v)  -> merge heads -> @ Wproj.T + bproj

Strategy: data-parallel over batch B=8 (one batch element per core).
Per core, scores are computed transposed (S^T[k, q]) per head so the
softmax numerator matmul (P^T stationary against V) needs no on-chip
transposes of the big P matrix.  Activations and weights are bf16 (host
converts) to halve HBM traffic; PSUM accumulation stays fp32.

Mask handling (host classifies the mask into 128x128 blocks):
  ONES block:  p = exp(s)                       (plain exp)
  TRIU block (upper-triangular in S^T layout, i.e. the causal diagonal):
               p = where(q >= k, exp(s), 1.0)   (one in-place affine_select)
  ZERO block:  p = 1  -> add colsum(V') once per qt via a rank-1 matmul
  MIXED block (general): p~ = (exp(s)-1)*m on DVE + the same colsum fix

The exp of the attention scores dominates the Scalar(ACT) engine, so
everything else is kept off it: projections/PV/transposes on PE, mask
selects + normalization + bias on Pool, reciprocals + half the copies on
DVE.  Emission is software-pipelined: QK-projection chunk i=0 first, the
remaining projections drained in small units between S windows, PV
lagging its head's exp by one head, and the output projection interleaved
with the last head's PV.
"""

import os
import sys

sys.path.insert(0, "/opt/trn_rl_repo")

import numpy as np

B, T, C = 8, 2048, 384
H, D = 6, 64
VW = D + 1  # per-head V width incl. the ones column (softmax denominator)
SCALE = float(C) ** -0.5
P = 128  # partitions / block edge
SW = 1024  # S-psum / exp window width
QCH = 512  # max moving width of one S matmul

ZERO, ONES, MIXED, TRIU = 0, 1, 2, 3

_CACHE = {}
LAST_PROFILE = {}


def _build_program(t, cls, mixed_ids, n_mixed, repeat=1):
    """Build the SPMD Bass program for sequence length t (t % P == 0).

    cls: [kb][qt] block classification.
    mixed_ids: dict (kb, qt) -> index into the packed mixed-mask input.
    repeat: if > 1, wrap the whole kernel in a hardware loop (timing only).
    """
    from contextlib import ExitStack

    import concourse.mybir as mybir
    import concourse.tile as tile
    from concourse import bacc
    from concourse.masks import make_identity

    f32 = mybir.dt.float32
    bf16 = mybir.dt.bfloat16

    nt = t // P  # number of 128-blocks along tokens

    nc = bacc.Bacc()

    xqT = nc.dram_tensor("xqT", [C, t], bf16, kind="ExternalInput")
    xkT = nc.dram_tensor("xkT", [C, t], bf16, kind="ExternalInput")
    xvT = nc.dram_tensor("xvT", [C, t], bf16, kind="ExternalInput")
    wkT = nc.dram_tensor("wkT", [C, C], bf16, kind="ExternalInput")
    wqT = nc.dram_tensor("wqT", [C, C], bf16, kind="ExternalInput")
    wvT = nc.dram_tensor("wvT", [C, C], bf16, kind="ExternalInput")
    wpT = nc.dram_tensor("wpT", [C, C], bf16, kind="ExternalInput")
    bpjh = nc.dram_tensor("bpjh", [1, C], bf16, kind="ExternalInput")
    if n_mixed:
        mmT = nc.dram_tensor("mmT", [n_mixed, P, P], bf16, kind="ExternalInput")
    y = nc.dram_tensor("y", [t, C], bf16, kind="ExternalOutput")
    c_dr = nc.dram_tensor("c_dr", [nt, H * VW], bf16, kind="Internal")

    # per-kb live q-range (multiples of 128); per-qt correction + live lists
    qlive = []
    for kb in range(nt):
        lv = [qt for qt in range(nt) if cls[kb][qt] != ZERO]
        qlive.append((lv[0] * P, (lv[-1] + 1) * P) if lv else None)
    corr_kbs = [
        [kb for kb in range(nt) if cls[kb][qt] in (ZERO, MIXED)] for qt in range(nt)
    ]
    live_kbs = [[kb for kb in range(nt) if cls[kb][qt] != ZERO] for qt in range(nt)]

    nca = nc  # alias for closures

    with ExitStack() as ctx:
        tc = ctx.enter_context(tile.TileContext(nc))
        if repeat > 1:
            ctx.enter_context(tc.For_i(0, repeat, 1))
        consts = ctx.enter_context(tc.tile_pool(name="consts", bufs=1))

        # ---- inputs: DMA issues spread across engine queues so the
        # critical xk/xq tiles are in SBUF as early as possible ----------
        w_s = {}
        for name, dram in (("wq", wqT), ("wk", wkT)):
            tl = consts.tile([P, 3, C], bf16, tag=f"w{name}", name=f"w_{name}")
            nc.sync.dma_start(
                out=tl, in_=dram[:, :].rearrange("(c p) f -> p c f", p=P)
            )
            w_s[name] = tl
        # x loads are split into [128, 512] column chunks, round-robined
        # over the three DMA-capable engine queues in consumption order
        # (xk/xq j-chunk 0 first) so the first S window starts ~3us in.
        xin = ctx.enter_context(tc.tile_pool(name="xin", bufs=1))
        xs = {}
        for nm, dram in (("xk", xkT), ("xq", xqT), ("xv", xvT)):
            for c in range(3):
                tl = xin.tile([P, t], bf16, tag=f"{nm}{c}", name=f"{nm}{c}")
                xs[(nm, c)] = tl
        dq = [nc.gpsimd, nc.sync]  # scalar issues would precede exps in
        di = 0  # ACT program order and delay them
        for j in range(t // QCH):
            order = (("xk", j), ("xq", j)) if j == 0 else (("xq", j), ("xk", j))
            for nm, jj in order:
                dram = xkT if nm == "xk" else xqT
                for c in range(3):
                    dq[di % 2].dma_start(
                        out=xs[(nm, c)][:, jj * QCH : (jj + 1) * QCH],
                        in_=dram[c * P : (c + 1) * P, jj * QCH : (jj + 1) * QCH],
                    )
                    di += 1
        for name, dram in (("wv", wvT), ("wp", wpT)):
            tl = consts.tile([P, 3, C], bf16, tag=f"w{name}", name=f"w_{name}")
            nc.sync.dma_start(
                out=tl, in_=dram[:, :].rearrange("(c p) f -> p c f", p=P)
            )
            w_s[name] = tl
        for c in range(3):  # xv consumed later (V' projection fillers)
            for j in range(t // QCH):
                dq[di % 2].dma_start(
                    out=xs[("xv", c)][:, j * QCH : (j + 1) * QCH],
                    in_=xvT[c * P : (c + 1) * P, j * QCH : (j + 1) * QCH],
                )
                di += 1
        bias_row = consts.tile([1, C], bf16, tag="bias_row", name="bias_row")
        nc.sync.dma_start(out=bias_row, in_=bpjh[:, :])
        ones_col = consts.tile([P, 1], bf16, tag="ones_col")
        nc.vector.memset(ones_col, 1.0)
        ones_row = consts.tile([1, P], bf16, tag="ones_row")
        nc.vector.memset(ones_row, 1.0)
        ident = consts.tile([P, P], bf16, tag="ident")
        make_identity(nc, ident)
        if n_mixed:
            mm_s = consts.tile([P, n_mixed, P], bf16, tag="mm")
            nc.scalar.dma_start(
                out=mm_s, in_=mmT[:, :, :].rearrange("n p f -> p n f")
            )

        # persistent activations: Q^T, K^T (feature-major), V' (token-major)
        qT_s = consts.tile([P, 3, t], bf16, tag="qT")
        kT_s = consts.tile([P, 3, t], bf16, tag="kT")
        vp_s = consts.tile([P, nt, H * VW], bf16, tag="vp")
        any_corr = any(corr_kbs[qt] for qt in range(nt))
        c_sb = c_big = None
        if any_corr:
            c_sb = consts.tile([1, nt, H * VW], bf16, tag="c_sb", name="c_sb")
            # 128-row broadcast of the corrections: one rank-1 matmul per qt
            # up front instead of one per (head, qt) inside the PV chains.
            c_big = consts.tile([P, nt, H * VW], bf16, tag="c_big", name="c_big")

        # ---- attention: S/exp per head (pipelined with PV of prev head) --
        obig = {}
        with ExitStack() as pools:
            obig_pool = pools.enter_context(tc.tile_pool(name="obig", bufs=nt))
            for qt in range(nt):
                obig[qt] = obig_pool.tile([P, C], bf16, tag="ob", name=f"obig{qt}")

            # one contiguous e-buffer per head: every kb's live q-range is
            # concatenated, so exp windows pack across kb boundaries
            # (fewer, full-width ACT instructions)
            pieces = []  # (kb, qlo, qhi, concat offset)
            etot = 0
            for kb in range(nt):
                if qlive[kb] is None:
                    continue
                qlo, qhi = qlive[kb]
                pieces.append((kb, qlo, qhi, etot))
                etot += qhi - qlo
            epool = pools.enter_context(tc.tile_pool(name="ebig", bufs=2))
            if n_mixed:
                pmix = pools.enter_context(
                    tc.tile_pool(name="pmix", bufs=max(4, min(n_mixed, nt * H)) + 2)
                )
                scr_pool = pools.enter_context(tc.tile_pool(name="scr", bufs=4))
            opool = pools.enter_context(
                tc.tile_pool(name="opsum", bufs=2, space="PSUM")
            )
            norm_pool = pools.enter_context(tc.tile_pool(name="norm", bufs=4))

            # ---- side-work units (emitted between S windows) ---------------
            side_stack = ExitStack()
            pproj = side_stack.enter_context(
                tc.tile_pool(name="pproj", bufs=2, space="PSUM")
            )
            # Pool/GPSIMD cannot access PSUM: PSUM-source copies go to DVE
            copy_eng = [nc.vector, nc.vector]

            def qk_unit(wname, dst, i, j, ceng=None):
                def emit():
                    ps = pproj.tile([P, QCH], f32, tag="pp")
                    src = "xk" if wname == "wq" else "xq"
                    for c in range(3):
                        nca.tensor.matmul(
                            ps[:, :QCH],
                            lhsT=w_s[wname][:, c, i * P : (i + 1) * P],
                            rhs=xs[(src, c)][:, j * QCH : (j + 1) * QCH],
                            start=(c == 0),
                            stop=(c == 2),
                        )
                    if ceng is nca.scalar:
                        ceng.activation(
                            out=dst[:, i, j * QCH : (j + 1) * QCH],
                            in_=ps[:, :QCH],
                            func=mybir.ActivationFunctionType.Copy,
                        )
                    else:
                        (ceng or copy_eng[(i + j) % 2]).tensor_copy(
                            out=dst[:, i, j * QCH : (j + 1) * QCH], in_=ps[:, :QCH]
                        )

                return emit

            def v_unit(tt):
                def emit():
                    ps = pproj.tile([P, QCH], f32, tag="pp")
                    for c in range(3):
                        nca.tensor.matmul(
                            ps[:, :C],
                            lhsT=xs[("xv", c)][:, tt * P : (tt + 1) * P],
                            rhs=w_s["wv"][:, c, :],
                            start=(c == 0),
                            stop=(c == 2),
                        )
                    copy_eng[tt % 2].tensor_copy(
                        out=vp_s[:, tt, :].rearrange("p (h w) -> p h w", h=H)[:, :, 0:D],
                        in_=ps[:, :C].rearrange("p (h d) -> p h d", h=H),
                    )
                    if tt == nt - 1:
                        nca.vector.memset(
                            vp_s.rearrange("p n (h w) -> p n h w", h=H)[
                                :, :, :, D : D + 1
                            ],
                            1.0,
                        )

                return emit

            def corr_unit():
                def emit():
                    # C[qt] = sum of colsum(V' block kb) over ZERO/MIXED kbs.
                    # Smallest-set-first; supersets keep accumulating (causal).
                    c_ps = pproj.tile([P, QCH], f32, tag="pp")
                    order = sorted(
                        (qt for qt in range(nt) if corr_kbs[qt]),
                        key=lambda q: len(corr_kbs[q]),
                    )
                    prev = None
                    for qt in order:
                        s = set(corr_kbs[qt])
                        if prev is not None and prev <= s:
                            add = sorted(s - prev)
                            fresh = False
                        else:
                            add = sorted(s)
                            fresh = True
                        for i2, kb in enumerate(add):
                            nca.tensor.matmul(
                                c_ps[0:1, : H * VW],
                                lhsT=ones_col,
                                rhs=vp_s[:, kb, :],
                                start=(fresh and i2 == 0),
                                stop=(i2 == len(add) - 1),
                                skip_group_check=True,
                            )
                        nca.vector.tensor_copy(
                            out=c_sb[:, qt, :], in_=c_ps[0:1, : H * VW]
                        )
                        prev = s
                        # broadcast the snapshot to all 128 q rows via a
                        # DRAM bounce (stride-0 partition reads need a
                        # DRAM source), costing DMA only — no PE/DVE work
                        nca.sync.dma_start(
                            out=c_dr[qt : qt + 1, :], in_=c_sb[:, qt, :]
                        )
                        nca.sync.dma_start(
                            out=c_big[:, qt, :],
                            in_=c_dr[qt : qt + 1, :].to_broadcast((P, H * VW)),
                        )

                return emit

            # fillers: small PE work units drained between S windows, paced so
            # the emitted PE work tracks the emitted ACT (exp) work.  Each
            # entry is (emit_fn, pe_ns_estimate).
            PE_NS = 1.0 / 2.4  # ns per moving row at full clock
            fillers = []
            for tt in range(nt):
                fillers.append((v_unit(tt), 3 * C * PE_NS + 60))
            if any_corr:
                n_corr_mm = sum(
                    1 for qt in range(nt) if corr_kbs[qt]
                )  # ~1 matmul per chain step for causal
                fillers.append((corr_unit(), n_corr_mm * H * VW * PE_NS + 1000))
            for i in range(1, 3):
                for j in range(t // QCH):
                    for wname, dst in (("wq", qT_s), ("wk", kT_s)):
                        fillers.append((qk_unit(wname, dst, i, j), 3 * QCH * PE_NS + 60))
            corr_done = nt + (1 if any_corr else 0)  # before first PV
            qk_done = {0: 0, 1: corr_done + 2 * (t // QCH), 2: len(fillers)}

            fill_pos = [0]
            budget = [0.0]  # ACT-emitted ns minus PE-emitted ns

            SLACK = 5000.0  # drain ahead of the ACT/PE balance point: real
            # PE throughput trails the row model (ldweights, psum-ring
            # waits), so early emission avoids bursts at head boundaries

            def drain(upto=None, balance=False, max_n=None):
                n = 0
                while fill_pos[0] < len(fillers):
                    if max_n is not None and n >= max_n:
                        break
                    fn, cost = fillers[fill_pos[0]]
                    if balance:
                        if budget[0] + SLACK < cost:
                            break
                    elif upto is not None and fill_pos[0] >= upto:
                        break
                    fn()
                    budget[0] -= cost
                    fill_pos[0] += 1
                    n += 1

            # ---- up-front QK projection for feature chunk i=0; the first S
            # window (head 0, kb 0) needs kT cols 0:128 and qT cols 0:512.
            # The first two units' copies go to the ACT engine: it is idle
            # until the first exp, which waits on exactly these copies, and
            # they precede every exp in ACT program order.
            qk_unit("wk", kT_s, 0, 0, ceng=nc.scalar)()
            qk_unit("wq", qT_s, 0, 0, ceng=nc.scalar)()
            qk_unit("wq", qT_s, 0, 1, ceng=nc.scalar)()
            for j in range(2, t // QCH):
                qk_unit("wq", qT_s, 0, j)()
            for j in range(1, t // QCH):
                qk_unit("wk", kT_s, 0, j)()


            sp_stack = ExitStack()
            spool = sp_stack.enter_context(
                tc.tile_pool(name="spsum", bufs=2, space="PSUM")
            )

            e_all = {}  # h -> {kb: (et, qlo)}
            p_all = {}  # h -> {(kb, qt): pt}

            def finish_kb(h, kb, qlo, off, et, p_tiles, after_kb):
                # mask fixes + tail hook once every column of kb's range
                # has been exp'd into the big e-buffer
                for qt in range(nt):
                    cl = cls[kb][qt]
                    sl_off = off + qt * P - qlo
                    if cl == TRIU:
                        sl = et[:, sl_off : sl_off + P]
                        nca.gpsimd.affine_select(
                            out=sl,
                            in_=sl,
                            pattern=[[1, P]],
                            channel_multiplier=-1,
                            base=0,
                            compare_op=mybir.AluOpType.is_ge,
                            fill=1.0,
                        )
                    elif cl == MIXED:
                        pt = pmix.tile([P, P], bf16, tag="pm")
                        sc = scr_pool.tile([P, 1], f32, tag="sc")
                        nca.vector.affine_mul_reduce(
                            out=pt,
                            accum_out=sc,
                            in0=et[:, sl_off : sl_off + P],
                            in1=mm_s[:, mixed_ids[(kb, qt)], :],
                            scale=1.0,
                            bias=-1.0,
                        )
                        p_tiles[(kb, qt)] = pt
                if after_kb is not None:
                    after_kb(kb)

            def emit_s_head(
                h, per_window_drain, sw=SW, spool_=None, after_kb=None, deadline=0
            ):
                # units below `deadline` must drain BY the head boundary;
                # spreading the forced part one-per-window avoids the burst
                # that overruns ACT's ~2-window exp backlog
                sp_pool = spool_ or spool
                hp = (h * D) // P
                ho = (h * D) % P
                e_tiles = {}
                p_tiles = {}
                e_all[h] = e_tiles
                p_all[h] = p_tiles
                if after_kb is not None:
                    for kb in range(nt):
                        if qlive[kb] is None:
                            after_kb(kb)
                et = epool.tile([P, etot], bf16, tag="eb", name=f"eb{h}")
                for kb, qlo, qhi, off in pieces:
                    # PV/select slices address et[:, qt*P - qlo_eff ...]
                    e_tiles[kb] = (et, qlo - off)
                bounds = []
                w = 0
                while w < etot:  # head 0 leads with two half windows so the
                    # first exp only waits on QK-proj unit j0
                    step = 512 if (h == 0 and sw == SW and w < 1024) else sw
                    bounds.append((w, min(w + step, etot)))
                    w += step
                for wi, (w0, w1) in enumerate(bounds):
                    sp = sp_pool.tile([P, sw], f32, tag="s", name=f"s_{h}_{w0}")
                    for kb, qlo, qhi, off in pieces:
                        p0, p1 = max(off, w0), min(off + qhi - qlo, w1)
                        if p0 >= p1:
                            continue
                        j0 = p0
                        while j0 < p1:  # stop at psum bank grid + piece end
                            j1 = min(p1, w0 + ((j0 - w0) // QCH + 1) * QCH)
                            q0 = qlo + (j0 - off)
                            nca.tensor.matmul(
                                sp[:, j0 - w0 : j1 - w0],
                                lhsT=kT_s[ho : ho + D, hp, kb * P : (kb + 1) * P],
                                rhs=qT_s[ho : ho + D, hp, q0 : q0 + (j1 - j0)],
                                start=True,
                                stop=True,
                            )
                            j0 = j1
                    nca.scalar.activation(
                        out=et[:, w0:w1],
                        in_=sp[:, : w1 - w0],
                        func=mybir.ActivationFunctionType.Exp,
                        scale=SCALE,
                    )
                    budget[0] += (w1 - w0) * (0.833 - PE_NS) + 185
                    per_window_drain()
                    left = min(deadline, len(fillers)) - fill_pos[0]
                    if left > 0 and left >= len(bounds) - wi - 1:
                        drain(upto=fill_pos[0] + 1)
                    for kb, qlo, qhi, off in pieces:
                        if w0 < off + (qhi - qlo) <= w1:  # kb completed here
                            finish_kb(h, kb, qlo, off, et, p_tiles, after_kb)

            def emit_pv(h, qt):
                e_tiles = e_all[h]
                p_tiles = p_all[h]
                op = opool.tile([P, VW], f32, tag="op")
                has_c = bool(corr_kbs[qt])
                nmm = len(live_kbs[qt])
                for i, kb in enumerate(live_kbs[qt]):
                    if cls[kb][qt] == MIXED:
                        lhsT = p_tiles[(kb, qt)][:]
                    else:
                        et, qlo = e_tiles[kb]
                        lhsT = et[:, qt * P - qlo : (qt + 1) * P - qlo]
                    nca.tensor.matmul(
                        op,
                        lhsT=lhsT,
                        rhs=vp_s[:, kb, h * VW : (h + 1) * VW],
                        start=(i == 0),
                        stop=(i == nmm - 1),
                    )
                if has_c:  # correction add moved off the PE onto DVE
                    tf = norm_pool.tile([P, VW], f32, tag="tf")
                    cb = c_big[:, qt, h * VW : (h + 1) * VW]
                    if nmm:
                        nca.vector.tensor_add(out=tf, in0=op, in1=cb)
                    else:  # fully-masked row of blocks (not causal)
                        nca.vector.tensor_copy(out=tf, in_=cb)
                    src = tf
                else:
                    src = op
                rc = norm_pool.tile([P, 1], f32, tag="rc")
                nca.vector.reciprocal(rc, src[:, D : D + 1])
                nca.vector.tensor_scalar_mul(
                    out=obig[qt][:, h * D : (h + 1) * D],
                    in0=src[:, 0:D],
                    scalar1=rc[:],
                )

            def per_window_drain():
                drain(balance=True, max_n=2)

            pv_end = {}
            for h in range(H - 1):
                hp = (h * D) // P
                drain(upto=qk_done[hp])  # S(h) needs QK chunk hp
                if h >= 2:  # e-ring slots of h-2 recycle during S(h)
                    drain(upto=pv_end[h - 2])
                if h >= 1:  # PV of the previous head drains inside S(h)
                    for qt in range(nt):
                        nmm = (1 if corr_kbs[qt] else 0) + len(live_kbs[qt])
                        fillers.append(
                            (
                                (lambda hh, qq: lambda: emit_pv(hh, qq))(h - 1, qt),
                                nmm * VW * PE_NS + 60,
                            )
                        )
                    pv_end[h - 1] = len(fillers)
                nxt_hp = min(((h + 1) * D) // P, 2)
                dl = max(qk_done[nxt_hp], pv_end.get(h - 1, 0))
                emit_s_head(h, per_window_drain, deadline=dl)
                e_all.pop(h - 3, None)
                p_all.pop(h - 3, None)
            drain(upto=len(fillers))  # flush side units + PV(0..3)
            # close the big S psum pool (LIFO: spool then pproj) and switch
            # to 512-wide windows (1 PSUM bank) so otp/yps fit alongside;
            # PV(5,kb) and the output projection run interleaved with head
            # 5's exp.
            sp_stack.close()
            side_stack.close()
            for qt in range(nt):  # PV of head 4, burst before head 5
                emit_pv(H - 2, qt)
            with ExitStack() as dpools:
                spool5 = dpools.enter_context(
                    tc.tile_pool(name="spsum5", bufs=2, space="PSUM")
                )
                otpool = dpools.enter_context(
                    tc.tile_pool(name="otp", bufs=2, space="PSUM")
                )
                ypool = dpools.enter_context(
                    tc.tile_pool(name="yp", bufs=2, space="PSUM")
                )
                ots_pool = dpools.enter_context(tc.tile_pool(name="otsp", bufs=3))
                ysb_pool = dpools.enter_context(tc.tile_pool(name="ysb", bufs=3))
                ots_q = {}

                def stage_b(qt):
                    otp = otpool.tile([P, C], bf16, tag="ot")
                    for c in range(3):
                        nc.tensor.transpose(
                            out=otp[:, c * P : (c + 1) * P],
                            in_=obig[qt][:, c * P : (c + 1) * P],
                            identity=ident,
                        )
                    ots = ots_pool.tile([P, C], bf16, tag="ots", name=f"ots{qt}")
                    if qt >= nt - 2:  # emitted after the last exp: ACT idle
                        nc.scalar.activation(
                            out=ots, in_=otp,
                            func=mybir.ActivationFunctionType.Copy,
                        )
                    else:
                        nc.vector.tensor_copy(out=ots, in_=otp)
                    ots_q[qt] = ots

                def stage_c(qt):
                    ots = ots_q.pop(qt)
                    yps = ypool.tile([P, C], f32, tag="y")
                    for c in range(3):
                        nc.tensor.matmul(
                            yps,
                            lhsT=ots[:, c * P : (c + 1) * P],
                            rhs=w_s["wp"][:, c, :],
                            start=(c == 0),
                            stop=False,
                        )
                    # bias as a rank-1 accumulation keeps the tail chain on
                    # the otherwise-idle PE/ACT instead of DVE
                    nc.tensor.matmul(
                        yps, lhsT=ones_row, rhs=bias_row, start=False, stop=True
                    )
                    ysb = ysb_pool.tile([P, C], bf16, tag="ysb")
                    if qt >= nt - 3:  # emitted after the last exp: ACT idle
                        nc.scalar.activation(
                            out=ysb, in_=yps,
                            func=mybir.ActivationFunctionType.Copy,
                        )
                    else:
                        nc.vector.tensor_copy(out=ysb, in_=yps)
                    yeng = (nc.sync, nc.gpsimd)[qt % 2]
                    yeng.dma_start(out=y[qt * P : (qt + 1) * P, :], in_=ysb)

                def tail_kb(kb):
                    emit_pv(H - 1, kb)
                    if kb - 1 >= 0:
                        stage_b(kb - 1)
                    if kb - 2 >= 0:
                        stage_c(kb - 2)

                emit_s_head(
                    H - 1,
                    lambda: None,
                    sw=512,
                    spool_=spool5,
                    after_kb=tail_kb,
                )
                stage_b(nt - 1)
                stage_c(nt - 2)
                stage_c(nt - 1)

        side_stack.close()

    nc.finalize()
    return nc


def _classify_mask(mask2d, t):
    """Host-side classification of the [t, t] 0/1 mask into 128x128 blocks.

    Returns cls[kb][qt] (ZERO/ONES/MIXED/TRIU), mixed block index map, and
    the packed transposed bf16 mixed blocks ([n, 128, 128], m^T layout)."""
    import ml_dtypes

    nt = t // P
    m = mask2d.reshape(nt, P, nt, P)  # [qt, qp, kb, kp]
    any_ = m.any(axis=(1, 3))  # [qt, kb]
    all_ = m.all(axis=(1, 3))
    triu = np.triu(np.ones((P, P), dtype=bool))  # m^T[k, q] = k <= q
    cls = [[ZERO] * nt for _ in range(nt)]
    mixed_ids = {}
    blocks = []
    for kb in range(nt):
        for qt in range(nt):
            if all_[qt, kb]:
                cls[kb][qt] = ONES
            elif any_[qt, kb]:
                blk = m[qt, :, kb, :].T  # [k, q]
                if np.array_equal(blk, triu):
                    cls[kb][qt] = TRIU
                else:
                    cls[kb][qt] = MIXED
                    mixed_ids[(kb, qt)] = len(blocks)
                    blocks.append(
                        np.ascontiguousarray(blk).astype(ml_dtypes.bfloat16)
                    )
    packed = np.stack(blocks) if blocks else None
    return cls, mixed_ids, packed


def kernel(query, key, value, mask, Wk, Wq, Wv, Wproj, bproj):
    import ml_dtypes

    from concourse.bass_utils import run_bass_kernel_spmd

    bf = ml_dtypes.bfloat16
    query = np.asarray(query, dtype=np.float32)
    key = np.asarray(key, dtype=np.float32)
    value = np.asarray(value, dtype=np.float32)
    b, t, c = query.shape
    mask2d = np.asarray(mask, dtype=np.int32).reshape(t, t) != 0

    cls, mixed_ids, packed = _classify_mask(mask2d, t)
    n_mixed = 0 if packed is None else len(packed)

    cache_key = (t, bytes(bytearray(v for row in cls for v in row)))
    if cache_key not in _CACHE:
        _CACHE[cache_key] = _build_program(t, cls, mixed_ids, n_mixed)
    nc = _CACHE[cache_key]

    wk = np.ascontiguousarray(np.asarray(Wk, np.float32).T).astype(bf)
    wq = np.ascontiguousarray(np.asarray(Wq, np.float32).T).astype(bf)
    wv = np.ascontiguousarray(np.asarray(Wv, np.float32).T).astype(bf)
    wp = np.ascontiguousarray(np.asarray(Wproj, np.float32).T).astype(bf)
    bp = np.asarray(bproj, np.float32).reshape(1, c).astype(bf)

    in_maps = []
    for i in range(b):
        m = {
            "xqT": np.ascontiguousarray(query[i].T).astype(bf),
            "xkT": np.ascontiguousarray(key[i].T).astype(bf),
            "xvT": np.ascontiguousarray(value[i].T).astype(bf),
            "wkT": wk,
            "wqT": wq,
            "wvT": wv,
            "wpT": wp,
            "bpjh": bp,
        }
        if n_mixed:
            m["mmT"] = packed
        in_maps.append(m)

    trace = bool(int(os.environ.get("BASS_MHA_TRACE", "0")))
    res = run_bass_kernel_spmd(nc, in_maps, core_ids=list(range(b)), trace=trace)
    LAST_PROFILE.clear()
    LAST_PROFILE.update(
        exec_time_ns=res.exec_time_ns,
        mean_exec_time_ns=res.mean_exec_time_ns,
        trace=res.instructions_and_trace,
    )
    return np.stack(
        [res.results[i]["y"].astype(np.float32) for i in range(b)]
    )

